# revision 28
# baseline (speedup 1.0000x reference)
"""Masked dot-product attention on 8 Trainium2 NeuronCores (Bass/Tile).

Problem: B=8, H=16, S=1024, D=64 attention where scores at key positions
k >= valid_lens[b] are masked to 1e-6 (not -inf) before softmax:
masked keys still contribute V with a uniform (unnormalized) weight of
exp(1e-6) ~= 1.

Sharding (SPMD, one program on 8 cores): each core takes 2 heads from EVERY
batch (core m gets heads b*16 + 2m, b*16 + 2m + 1). Since the masked length
is per-batch, every core sees the identical per-slot workload vector
[C_0, C_0, C_1, C_1, ..., C_7, C_7] where C_b = min(8, L_b//128 + 1) is the
number of 128-row key chunks that must be computed densely. The program is
specialized to that vector (compile cached per distinct valid_lens).

Masking, exactly:
  - kT rows with k >= L are zeroed on the host: their scores become exactly 0
    and their unnormalized weight exp(0) = 1 (vs exp(1e-6) in the reference:
    rel diff 1e-6, far below fp32 tolerance).
  - chunks >= C_b are skipped entirely; every skipped row would have weight
    exactly 1, so the host folds sum_{k >= C_b*128} [V[k], 1] into the
    (always masked) last row of the boundary chunk's V_aug. This is exact.

Device pipeline per head slot (fp32; matmuls in fp32r = full PE rate at
free dim >= 256, ~1.6e-4 max rel err measured on HW):
  1. scoresT[k, q] = K @ Q^T per 128-key chunk, as TWO concurrent row-tiled
     matmuls: query half 0 lives on SBUF partitions 0:64 feeding PE rows
     0:63, half 1 on partitions 64:128 feeding rows 64:127 (the contraction
     dim D=64 only fills half the array, so both halves run in parallel —
     measured 291 ns per pair vs 431 ns for one serial matmul). K chunks are
     duplicated across both partition halves (small); Q is not duplicated.
  2. pT = exp(0.125 * scoresT)  (ACT, PSUM->SBUF, scale folded in; the ACT
     engine at 1 elem/lane/cycle is the pipeline's bottleneck engine)
  3. outT[d(+1), q] += V_aug[kc].T @ pT[kc]   (ones-column of V_aug makes
     row 64 the softmax denominator for free), lagging exp by `lag` chunks
     so the PE never blocks the ACT stream
  4. PE-transpose outT to [q, d+1]; DVE reciprocal + per-partition scale; DMA.
"""

from contextlib import ExitStack

import numpy as np

import concourse.bass as bass  # noqa: F401
import concourse.mybir as mybir
import concourse.tile as tile
from concourse import bacc
from concourse.masks import make_identity

F32 = mybir.dt.float32
F32R = mybir.dt.float32r

B, H, S, D = 8, 16, 1024, 64
N_CORES = 8
HPC = H // N_CORES     # heads per (core, batch) = 2
KC = S // 128          # key chunks per full head
QH = S // 512          # query halves
EXPF = mybir.ActivationFunctionType.Exp
SCALE = 1.0 / 8.0      # 1/sqrt(64)

DENSE_CVEC = (KC,) * B

# Tunables (experiment knobs; values are compile-time).
CFG = {
    "lag": 3,          # chunks between exp and its PV consumption
    "pack": "qsplit",  # QK row-packing: qh0 on partitions 0:64, qh1 on 64:128
    "qt_dup": True,
    "pt_bufs": 6,
    "ps_s_bufs": 2,
}


def _emit_head(nc, pools, qT, kT, v, out, h, C, pending):
    """Emit one head slot with C dense key chunks. `pending` holds deferred
    epilogues (PE transposes) of previous heads, flushed after this head's
    early QK work so the PE never stalls on the DVE evacuation."""
    (qk_pool, va_pool, pt_pool, pv_pool, ob_pool, sc_pool,
     ps_s_pool, ps_o_pool, ps_t_pool, identity) = pools

    pack = CFG["pack"]
    nE = (C + 1) // 2 if pack is True else C
    nO = C // 2 if pack is True else 0
    if pack == "qsplit":
        # qh0 on partitions 0:64, qh1 on 64:128 — Q is NOT duplicated; the
        # (small) K is duplicated instead. Each chunk = 2 concurrent row-tile
        # MMs writing the two banks of its score tile.
        qt = qk_pool.tile([128, 512], F32R, tag="qt")
        nc.sync.dma_start(qt[0:64, :], qT[h][:, 0:512])
        nc.sync.dma_start(qt[64:128, :], qT[h][:, 512:1024])
        kt = qk_pool.tile([128, C * 128], F32R, tag="kt")
        nc.sync.dma_start(kt[0:64, :], kT[h][:, 0:C * 128])
        nc.sync.dma_start(kt[64:128, :], kT[h][:, 0:C * 128])
    elif pack:
        qt = qk_pool.tile([128, S], F32R, tag="qt")
        for qh in range(QH):
            sl = slice(qh * 512, (qh + 1) * 512)
            nc.sync.dma_start(qt[0:64, sl], qT[h][:, sl])
            nc.sync.dma_start(qt[64:128, sl], qT[h][:, sl])
        kt = qk_pool.tile([128, nE * 128], F32R, tag="kt")
        nc.sync.dma_start(
            kt[0:64, :],
            kT[h][:, 0:nE * 256].rearrange("d (i two c) -> d two i c",
                                           two=2, c=128)[:, 0, :, :]
            if C > 1 else kT[h][:, 0:128])
        if nO:
            nc.sync.dma_start(
                kt[64:128, 0:nO * 128],
                kT[h][:, 0:nO * 256].rearrange("d (i two c) -> d two i c",
                                               two=2, c=128)[:, 1, :, :])
    else:
        qt = qk_pool.tile([64, S], F32R, tag="qt")
        for qh in range(QH):
            sl = slice(qh * 512, (qh + 1) * 512)
            nc.sync.dma_start(qt[:, sl], qT[h][:, sl])
        kt = qk_pool.tile([64, C * 128], F32R, tag="kt")
        nc.sync.dma_start(kt[:], kT[h][:, 0:C * 128])
    va = va_pool.tile([128, C, D + 1], F32R, tag="va")
    nc.sync.dma_start(
        va[:], v[h][0:C * 128].rearrange("(kc p) d -> p kc d", p=128))

    ps_o = [ps_o_pool.tile([D + 1, 512], F32, tag="ps_o", name=f"ps_o{qh}")
            for qh in range(QH)]

    def emit_qk_exp(kc):
        """Packed pair (kc, kc+1) when kc even and kc+1 < C; else solo kc."""
        i = kc // 2
        tiles = []
        ps_a = ps_s_pool.tile([128, S], F32, tag="ps_s", name="ps_sa")
        tiles.append(ps_a)
        if pack == "qsplit":
            nc.tensor.matmul(
                ps_a[:, 0:512],
                lhsT=kt[0:64, kc * 128:(kc + 1) * 128],
                rhs=qt[0:64, :],
                start=True, stop=True,
            )
            nc.tensor.matmul(
                ps_a[:, 512:1024],
                lhsT=kt[64:128, kc * 128:(kc + 1) * 128],
                rhs=qt[64:128, :],
                start=True, stop=True,
            )
            pt = pt_pool.tile([128, S], F32R, tag="pt", name="pt0")
            nc.scalar.activation(pt[:], ps_a[:], EXPF, scale=SCALE)
            return [(kc, pt)]
        if not pack:
            for qh in range(QH):
                nc.tensor.matmul(
                    ps_a[:, qh * 512:(qh + 1) * 512],
                    lhsT=kt[:, kc * 128:(kc + 1) * 128],
                    rhs=qt[:, qh * 512:(qh + 1) * 512],
                    start=True, stop=True,
                )
            pt = pt_pool.tile([128, S], F32R, tag="pt", name="pt0")
            nc.scalar.activation(pt[:], ps_a[:], EXPF, scale=SCALE)
            return [(kc, pt)]
        paired = kc + 1 < C
        if paired:
            ps_b = ps_s_pool.tile([128, S], F32, tag="ps_s", name="ps_sb")
            tiles.append(ps_b)
        for qh in range(QH):
            nc.tensor.matmul(
                ps_a[:, qh * 512:(qh + 1) * 512],
                lhsT=kt[0:64, i * 128:(i + 1) * 128],
                rhs=qt[0:64, qh * 512:(qh + 1) * 512],
                start=True, stop=True,
            )
            if paired:
                nc.tensor.matmul(
                    ps_b[:, qh * 512:(qh + 1) * 512],
                    lhsT=kt[64:128, i * 128:(i + 1) * 128],
                    rhs=qt[64:128, qh * 512:(qh + 1) * 512],
                    start=True, stop=True,
                )
        out = []
        for j, ps in enumerate(tiles):
            pt = pt_pool.tile([128, S], F32R, tag="pt", name=f"pt{j}")
            nc.scalar.activation(pt[:], ps[:], EXPF, scale=SCALE)
            out.append((kc + j, pt))
        return out

    pending_pv = []
    first = True
    kc = 0
    while kc < C:
        produced = emit_qk_exp(kc)
        kc += len(produced)
        if first:
            while pending:
                pending.pop(0)()
            first = False
        pending_pv.extend(produced)
        while len(pending_pv) > CFG["lag"]:
            c0, pt0 = pending_pv.pop(0)
            _emit_pv(nc, ps_o, va, pt0, c0, C)
    for c0, pt0 in pending_pv:
        _emit_pv(nc, ps_o, va, pt0, c0, C)

    # Evacuate PSUM on the DVE right away; defer the PE work.
    pvs = []
    for qh in range(QH):
        pv_sb = pv_pool.tile([D + 1, 512], F32, tag="pv")
        nc.vector.tensor_copy(pv_sb[:], ps_o[qh][:])
        pvs.append(pv_sb)

    def epilogue():
        for qh in range(QH):
            ps_t = ps_t_pool.tile([128, 4, D + 1], F32, tag="ps_t")
            for j in range(4):
                nc.tensor.transpose(
                    ps_t[:, j, :],
                    pvs[qh][:, j * 128:(j + 1) * 128],
                    identity[0:D + 1, 0:D + 1],
                )
            recip = sc_pool.tile([128, 4], F32, tag="recip")
            nc.vector.reciprocal(recip[:], ps_t[:, :, D])
            ob = ob_pool.tile([128, 4, D], F32, tag="ob")
            for j in range(4):
                nc.vector.tensor_scalar_mul(
                    ob[:, j, :], ps_t[:, j, 0:D], recip[:, j:j + 1])
            nc.sync.dma_start(
                out[h][qh * 512:(qh + 1) * 512, :].rearrange(
                    "(j p) d -> p j d", p=128),
                ob[:],
            )

    pending.append(epilogue)


def _emit_pv(nc, ps_o, va, pt, kc, C):
    for qh in range(QH):
        nc.tensor.matmul(
            ps_o[qh][:],
            lhsT=va[:, kc, :],
            rhs=pt[:, qh * 512:(qh + 1) * 512],
            start=(kc == 0), stop=(kc == C - 1),
        )


def build_program(cvec=DENSE_CVEC, loop: int = 1, repeat: int = 1):
    """One SPMD program; head slot s (0..15) covers batch s//2 with
    cvec[s//2] dense chunks."""
    nc = bacc.Bacc("TRN2", target_bir_lowering=False, debug=False,
                   enable_asserts=True, num_devices=N_CORES)
    qT = nc.dram_tensor("qT", [H, D, S], F32R, kind="ExternalInput").ap()
    kT = nc.dram_tensor("kT", [H, D, S], F32R, kind="ExternalInput").ap()
    v = nc.dram_tensor("v", [H, S, D + 1], F32R, kind="ExternalInput").ap()
    out = nc.dram_tensor("out", [H, S, D], F32, kind="ExternalOutput").ap()

    with tile.TileContext(nc) as tc:
        with ExitStack() as ctx:
            const_pool = ctx.enter_context(tc.tile_pool(name="const", bufs=1))
            identity = const_pool.tile([128, 128], F32)
            make_identity(nc, identity[:])

            pools = (
                ctx.enter_context(tc.tile_pool(name="qk", bufs=3)),
                ctx.enter_context(tc.tile_pool(name="va", bufs=3)),
                ctx.enter_context(tc.tile_pool(name="pt", bufs=CFG["pt_bufs"])),
                ctx.enter_context(tc.tile_pool(name="pv", bufs=6)),
                ctx.enter_context(tc.tile_pool(name="ob", bufs=4)),
                ctx.enter_context(tc.tile_pool(name="sc", bufs=6)),
                ctx.enter_context(tc.tile_pool(name="ps_s", bufs=CFG["ps_s_bufs"], space="PSUM")),
                ctx.enter_context(tc.tile_pool(name="ps_o", bufs=2, space="PSUM")),
                ctx.enter_context(tc.tile_pool(name="ps_t", bufs=2, space="PSUM")),
                identity,
            )

            plan = slot_plan(cvec)

            def body(_i=None):
                pending = []
                for _ in range(repeat):
                    for h in range(H):
                        _emit_head(nc, pools, qT, kT, v, out, h,
                                   cvec[plan[h]], pending)
                while pending:
                    pending.pop(0)()

            if loop == 1:
                body()
            else:
                with tc.For_i(0, loop, 1):
                    body()
    nc.compile()
    return nc


def cvec_of(valid_lens):
    vl = np.asarray(valid_lens).astype(np.int64).reshape(B)
    return tuple(int(min(KC, L // 128 + 1)) for L in vl)


def slot_plan(cvec):
    """Per-core slot order: batch ids (each appearing HPC times), heavy and
    light heads interleaved so small heads' serial chains hide under big
    neighbors' ACT backlog. Deterministic in cvec (host and device agree)."""
    pairs = sorted([(cvec[b], b) for b in range(B) for _ in range(HPC)],
                   key=lambda x: (-x[0], x[1]))
    last = pairs.pop()[1]  # smallest head last: shortest serial drain tail
    order = []
    lo, hi = 0, len(pairs) - 1
    while lo <= hi:
        order.append(pairs[lo][1])
        lo += 1
        if lo <= hi:
            order.append(pairs[hi][1])
            hi -= 1
    order.append(last)
    return order


def make_in_maps(queries, keys, values, valid_lens):
    """Per-core inputs: core m's head slot 2b+j holds head (b, 2m+j)."""
    q = np.ascontiguousarray(
        np.asarray(queries, dtype=np.float32)).reshape(B, H, S, D)
    k = np.ascontiguousarray(
        np.asarray(keys, dtype=np.float32)).reshape(B, H, S, D)
    v = np.ascontiguousarray(
        np.asarray(values, dtype=np.float32)).reshape(B, H, S, D)
    vl = np.asarray(valid_lens).astype(np.int64).reshape(B)
    cvec = cvec_of(vl)

    # [B, H, D+1, ...] staging with mask + fold applied per batch.
    km = k.copy()
    va = np.empty((B, H, S, D + 1), np.float32)
    va[..., :D] = v
    va[..., D] = 1.0
    for b in range(B):
        L, C = int(vl[b]), cvec[b]
        km[b, :, L:, :] = 0.0
        if C < KC:
            # Skipped rows all have unnormalized weight exactly 1; fold their
            # V_aug sum into the (masked) last row of the boundary chunk.
            va[b, :, C * 128 - 1, :] += va[b, :, C * 128:, :].sum(axis=1)

    qT = q.transpose(0, 1, 3, 2)   # [B, H, D, S]
    kT = km.transpose(0, 1, 3, 2)

    # slot s of core m holds head (plan[s], 2m + j) where j counts prior
    # occurrences of plan[s] in the plan.
    plan = slot_plan(cvec)
    occ = {}
    slot_heads = []  # (batch, j) per slot
    for b in plan:
        j = occ.get(b, 0)
        occ[b] = j + 1
        slot_heads.append((b, j))

    in_maps = []
    for m in range(N_CORES):
        idx = ([], [])
        for b, j in slot_heads:
            idx[0].append(b)
            idx[1].append(2 * m + j)
        in_maps.append({
            "qT": np.ascontiguousarray(qT[idx[0], idx[1]]),
            "kT": np.ascontiguousarray(kT[idx[0], idx[1]]),
            "v": np.ascontiguousarray(va[idx[0], idx[1]]),
        })
    return in_maps, cvec


def scatter_outputs(results, cvec):
    """Inverse of the head assignment: full [B*H, S, D] from per-core outs."""
    plan = slot_plan(cvec)
    occ = {}
    slot_heads = []
    for b in plan:
        j = occ.get(b, 0)
        occ[b] = j + 1
        slot_heads.append((b, j))
    out = np.empty((B, H, S, D), dtype=np.float32)
    for m in range(N_CORES):
        for s, (b, j) in enumerate(slot_heads):
            out[b, 2 * m + j] = results[m][s]
    return out.reshape(B * H, S, D)


_NC_CACHE = {}


def _get_nc(cvec, loop=1, repeat=1):
    key = (cvec, loop, repeat, tuple(sorted(CFG.items())))
    if key not in _NC_CACHE:
        _NC_CACHE[key] = build_program(cvec, loop, repeat)
    return _NC_CACHE[key]


def kernel(queries, keys, values, valid_lens):
    from concourse.bass_utils import run_bass_kernel_spmd

    in_maps, cvec = make_in_maps(queries, keys, values, valid_lens)
    nc = _get_nc(cvec)
    res = run_bass_kernel_spmd(nc, in_maps, list(range(N_CORES)))
    return scatter_outputs(
        [res.results[m]["out"] for m in range(N_CORES)], cvec)


# ----------------------------------------------------------------------------
# Cached jitted runner (used by test.py for timing; avoids per-call re-trace
# and ships inputs to the devices once).
# ----------------------------------------------------------------------------
_RUNNER_CACHE = {}


def _get_runner(cvec=DENSE_CVEC, loop: int = 1):
    key = (cvec, loop, tuple(sorted(CFG.items())))
    if key in _RUNNER_CACHE:
        return _RUNNER_CACHE[key]

    import jax
    from jax.sharding import Mesh, PartitionSpec, NamedSharding
    from jax.experimental.shard_map import shard_map
    from concourse import bass2jax

    nc = _get_nc(cvec, loop)
    bass2jax.install_neuronx_cc_hook()

    partition_name = (nc.partition_id_tensor.name
                      if nc.partition_id_tensor else None)
    in_names, out_names, out_avals, zero_outs = [], [], [], []
    for alloc in nc.m.functions[0].allocations:
        if not isinstance(alloc, mybir.MemoryLocationSet):
            continue
        name = alloc.memorylocations[0].name
        if alloc.kind == "ExternalInput":
            if name != partition_name:
                in_names.append(name)
        elif alloc.kind == "ExternalOutput":
            out_names.append(name)
            shape = tuple(alloc.tensor_shape)
            dtype = mybir.dt.np(alloc.dtype)
            out_avals.append(jax.core.ShapedArray(shape, dtype))
            zero_outs.append(np.zeros(shape, dtype))
    n_params = len(in_names)
    n_outs = len(out_avals)
    all_in_names = in_names + out_names
    if partition_name is not None:
        all_in_names = all_in_names + [partition_name]

    def _body(*args):
        operands = list(args)
        if partition_name is not None:
            operands.append(bass2jax.partition_id_tensor())
        outs = bass2jax._bass_exec_p.bind(
            *operands,
            out_avals=tuple(out_avals),
            in_names=tuple(all_in_names),
            out_names=tuple(out_names),
            lowering_input_output_aliases=(),
            sim_require_finite=True,
            sim_require_nnan=True,
            nc=nc,
        )
        return tuple(outs)

    devices = jax.devices()[:N_CORES]
    mesh = Mesh(np.asarray(devices), ("core",))
    donate = tuple(range(n_params, n_params + n_outs))
    sharded = jax.jit(
        shard_map(
            _body, mesh=mesh,
            in_specs=(PartitionSpec("core"),) * (n_params + n_outs),
            out_specs=(PartitionSpec("core"),) * n_outs,
            check_rep=False,
        ),
        donate_argnums=donate, keep_unused=True,
    )

    def run(in_maps):
        concat_in = [
            np.concatenate([m[name] for m in in_maps], axis=0)
            for name in in_names
        ]
        concat_zeros = [
            np.zeros((N_CORES * z.shape[0], *z.shape[1:]), z.dtype)
            for z in zero_outs
        ]
        out_arrs = sharded(*concat_in, *concat_zeros)
        return [
            {
                name: np.asarray(out_arrs[i]).reshape(
                    N_CORES, *out_avals[i].shape)[c]
                for i, name in enumerate(out_names)
            }
            for c in range(N_CORES)
        ]

    def make_dev_args(in_maps):
        sh = NamedSharding(mesh, PartitionSpec("core"))
        concat_in = [
            np.concatenate([m[name] for m in in_maps], axis=0)
            for name in in_names
        ]
        dev_in = [jax.device_put(a, sh) for a in concat_in]
        jax.block_until_ready(dev_in)

        def fresh_zeros():
            zs = [jax.device_put(
                np.zeros((N_CORES * z.shape[0], *z.shape[1:]), z.dtype), sh)
                for z in zero_outs]
            jax.block_until_ready(zs)
            return zs

        return dev_in, fresh_zeros

    _RUNNER_CACHE[key] = (run, sharded, make_dev_args, out_names, out_avals, nc)
    return _RUNNER_CACHE[key]


# revision 30
# speedup vs baseline: 1.4564x; 1.4564x over previous
"""Masked dot-product attention on 8 Trainium2 NeuronCores (Bass/Tile).

Problem: B=8, H=16, S=1024, D=64 attention where scores at key positions
k >= valid_lens[b] are masked to 1e-6 (not -inf) before softmax:
masked keys still contribute V with a uniform (unnormalized) weight of
exp(1e-6) ~= 1.

Sharding (SPMD, one program on 8 cores): each core takes 2 heads from EVERY
batch (core m gets heads b*16 + 2m, b*16 + 2m + 1). Since the masked length
is per-batch, every core sees the identical per-slot workload vector
[C_0, C_0, C_1, C_1, ..., C_7, C_7] where C_b = min(8, L_b//128 + 1) is the
number of 128-row key chunks that must be computed densely. The program is
specialized to that vector (compile cached per distinct valid_lens).

Masking, exactly:
  - kT rows with k >= L are zeroed on the host: their scores become exactly 0
    and their unnormalized weight exp(0) = 1 (vs exp(1e-6) in the reference:
    rel diff 1e-6, far below fp32 tolerance).
  - chunks >= C_b are skipped entirely; every skipped row would have weight
    exactly 1, so the host folds sum_{k >= C_b*128} [V[k], 1] into the
    (always masked) last row of the boundary chunk's V_aug. This is exact.

Device pipeline per head slot (fp32; matmuls in fp32r = full PE rate at
free dim >= 256, ~1.6e-4 max rel err measured on HW):
  1. scoresT[k, q] = K @ Q^T per 128-key chunk, as TWO concurrent row-tiled
     matmuls: query half 0 lives on SBUF partitions 0:64 feeding PE rows
     0:63, half 1 on partitions 64:128 feeding rows 64:127 (the contraction
     dim D=64 only fills half the array, so both halves run in parallel —
     measured 291 ns per pair vs 431 ns for one serial matmul). K chunks are
     duplicated across both partition halves (small); Q is not duplicated.
  2. pT = exp(0.125 * scoresT)  (ACT, PSUM->SBUF, scale folded in; the ACT
     engine at 1 elem/lane/cycle is the pipeline's bottleneck engine)
  3. outT[d(+1), q] += V_aug[kc].T @ pT[kc]   (ones-column of V_aug makes
     row 64 the softmax denominator for free), lagging exp by `lag` chunks
     so the PE never blocks the ACT stream
  4. PE-transpose outT to [q, d+1]; DVE reciprocal + per-partition scale; DMA.
"""

from contextlib import ExitStack

import numpy as np

import concourse.bass as bass  # noqa: F401
import concourse.mybir as mybir
import concourse.tile as tile
from concourse import bacc
from concourse.masks import make_identity

F32 = mybir.dt.float32
F32R = mybir.dt.float32r

B, H, S, D = 8, 16, 1024, 64
N_CORES = 8
HPC = H // N_CORES     # heads per (core, batch) = 2
KC = S // 128          # key chunks per full head
QH = S // 512          # query halves
EXPF = mybir.ActivationFunctionType.Exp
SCALE = 1.0 / 8.0      # 1/sqrt(64)

DENSE_CVEC = (KC,) * B

# Tunables (experiment knobs; values are compile-time).
CFG = {
    "lag": 3,          # chunks between exp and its PV consumption
    "pack": "qsplit",  # QK row-packing: qh0 on partitions 0:64, qh1 on 64:128
    "qt_dup": True,
    "pt_bufs": 6,
    "ps_s_bufs": 2,
    "ps_o_bufs": 2,
    "ps_t_bufs": 2,
    "qk_bufs": 3,
    "va_bufs": 3,
    "epi": "pe",       # "pe": PE-transpose epilogue -> out [S, D]
                       # "bcast": gpsimd broadcast divide -> out [D, S],
                       #          transposed on the host during unshard
}


def _emit_head(nc, pools, qT, kT, v, out, h, C, pending):
    """Emit one head slot with C dense key chunks. `pending` holds deferred
    epilogues (PE transposes) of previous heads, flushed after this head's
    early QK work so the PE never stalls on the DVE evacuation."""
    (qk_pool, va_pool, pt_pool, pv_pool, ob_pool, sc_pool,
     ps_s_pool, ps_o_pool, ps_t_pool, identity) = pools

    pack = CFG["pack"]
    nE = (C + 1) // 2 if pack is True else C
    nO = C // 2 if pack is True else 0
    if pack == "qsplit":
        # qh0 on partitions 0:64, qh1 on 64:128 — Q is NOT duplicated; the
        # (small) K is duplicated instead. Each chunk = 2 concurrent row-tile
        # MMs writing the two banks of its score tile.
        qt = qk_pool.tile([128, 512], F32R, tag="qt")
        nc.sync.dma_start(qt[0:64, :], qT[h][:, 0:512])
        nc.sync.dma_start(qt[64:128, :], qT[h][:, 512:1024])
        kt = qk_pool.tile([128, C * 128], F32R, tag="kt")
        nc.sync.dma_start(kt[0:64, :], kT[h][:, 0:C * 128])
        nc.sync.dma_start(kt[64:128, :], kT[h][:, 0:C * 128])
    elif pack:
        qt = qk_pool.tile([128, S], F32R, tag="qt")
        for qh in range(QH):
            sl = slice(qh * 512, (qh + 1) * 512)
            nc.sync.dma_start(qt[0:64, sl], qT[h][:, sl])
            nc.sync.dma_start(qt[64:128, sl], qT[h][:, sl])
        kt = qk_pool.tile([128, nE * 128], F32R, tag="kt")
        nc.sync.dma_start(
            kt[0:64, :],
            kT[h][:, 0:nE * 256].rearrange("d (i two c) -> d two i c",
                                           two=2, c=128)[:, 0, :, :]
            if C > 1 else kT[h][:, 0:128])
        if nO:
            nc.sync.dma_start(
                kt[64:128, 0:nO * 128],
                kT[h][:, 0:nO * 256].rearrange("d (i two c) -> d two i c",
                                               two=2, c=128)[:, 1, :, :])
    else:
        qt = qk_pool.tile([64, S], F32R, tag="qt")
        for qh in range(QH):
            sl = slice(qh * 512, (qh + 1) * 512)
            nc.sync.dma_start(qt[:, sl], qT[h][:, sl])
        kt = qk_pool.tile([64, C * 128], F32R, tag="kt")
        nc.sync.dma_start(kt[:], kT[h][:, 0:C * 128])
    va = va_pool.tile([128, C, D + 1], F32R, tag="va")
    nc.sync.dma_start(
        va[:], v[h][0:C * 128].rearrange("(kc p) d -> p kc d", p=128))

    ps_o = [ps_o_pool.tile([D + 1, 512], F32, tag="ps_o", name=f"ps_o{qh}")
            for qh in range(QH)]

    def emit_qk_exp(kc):
        """Packed pair (kc, kc+1) when kc even and kc+1 < C; else solo kc."""
        i = kc // 2
        tiles = []
        ps_a = ps_s_pool.tile([128, S], F32, tag="ps_s", name="ps_sa")
        tiles.append(ps_a)
        if pack == "qsplit":
            nc.tensor.matmul(
                ps_a[:, 0:512],
                lhsT=kt[0:64, kc * 128:(kc + 1) * 128],
                rhs=qt[0:64, :],
                start=True, stop=True,
            )
            nc.tensor.matmul(
                ps_a[:, 512:1024],
                lhsT=kt[64:128, kc * 128:(kc + 1) * 128],
                rhs=qt[64:128, :],
                start=True, stop=True,
            )
            pt = pt_pool.tile([128, S], F32R, tag="pt", name="pt0")
            nc.scalar.activation(pt[:], ps_a[:], EXPF, scale=SCALE)
            return [(kc, pt)]
        if not pack:
            for qh in range(QH):
                nc.tensor.matmul(
                    ps_a[:, qh * 512:(qh + 1) * 512],
                    lhsT=kt[:, kc * 128:(kc + 1) * 128],
                    rhs=qt[:, qh * 512:(qh + 1) * 512],
                    start=True, stop=True,
                )
            pt = pt_pool.tile([128, S], F32R, tag="pt", name="pt0")
            nc.scalar.activation(pt[:], ps_a[:], EXPF, scale=SCALE)
            return [(kc, pt)]
        paired = kc + 1 < C
        if paired:
            ps_b = ps_s_pool.tile([128, S], F32, tag="ps_s", name="ps_sb")
            tiles.append(ps_b)
        for qh in range(QH):
            nc.tensor.matmul(
                ps_a[:, qh * 512:(qh + 1) * 512],
                lhsT=kt[0:64, i * 128:(i + 1) * 128],
                rhs=qt[0:64, qh * 512:(qh + 1) * 512],
                start=True, stop=True,
            )
            if paired:
                nc.tensor.matmul(
                    ps_b[:, qh * 512:(qh + 1) * 512],
                    lhsT=kt[64:128, i * 128:(i + 1) * 128],
                    rhs=qt[64:128, qh * 512:(qh + 1) * 512],
                    start=True, stop=True,
                )
        out = []
        for j, ps in enumerate(tiles):
            pt = pt_pool.tile([128, S], F32R, tag="pt", name=f"pt{j}")
            nc.scalar.activation(pt[:], ps[:], EXPF, scale=SCALE)
            out.append((kc + j, pt))
        return out

    pending_pv = []
    first = True
    kc = 0
    while kc < C:
        produced = emit_qk_exp(kc)
        kc += len(produced)
        if first:
            while pending:
                pending.pop(0)()
            first = False
        pending_pv.extend(produced)
        while len(pending_pv) > CFG["lag"]:
            c0, pt0 = pending_pv.pop(0)
            _emit_pv(nc, ps_o, va, pt0, c0, C)
    for c0, pt0 in pending_pv:
        _emit_pv(nc, ps_o, va, pt0, c0, C)

    # Evacuate PSUM on the DVE right away; defer the PE work.
    pvs = []
    for qh in range(QH):
        pv_sb = pv_pool.tile([D + 1, 512], F32, tag="pv")
        nc.vector.tensor_copy(pv_sb[:], ps_o[qh][:])
        pvs.append(pv_sb)

    def epilogue():
        if CFG["epi"] == "bcast":
            for qh in range(QH):
                nc.vector.reciprocal(pvs[qh][D:D + 1, :], pvs[qh][D:D + 1, :])
                rb = ob_pool.tile([64, 512], F32, tag="rb")
                nc.gpsimd.partition_broadcast(rb[:], pvs[qh][D:D + 1, :])
                ot = ob_pool.tile([64, 512], F32, tag="ot")
                nc.vector.tensor_mul(ot[:], pvs[qh][0:D, :], rb[:])
                nc.sync.dma_start(out[h][:, qh * 512:(qh + 1) * 512], ot[:])
            return
        for qh in range(QH):
            ps_t = ps_t_pool.tile([128, 4, D + 1], F32, tag="ps_t")
            for j in range(4):
                nc.tensor.transpose(
                    ps_t[:, j, :],
                    pvs[qh][:, j * 128:(j + 1) * 128],
                    identity[0:D + 1, 0:D + 1],
                )
            recip = sc_pool.tile([128, 4], F32, tag="recip")
            nc.vector.reciprocal(recip[:], ps_t[:, :, D])
            ob = ob_pool.tile([128, 4, D], F32, tag="ob")
            for j in range(4):
                nc.vector.tensor_scalar_mul(
                    ob[:, j, :], ps_t[:, j, 0:D], recip[:, j:j + 1])
            nc.sync.dma_start(
                out[h][qh * 512:(qh + 1) * 512, :].rearrange(
                    "(j p) d -> p j d", p=128),
                ob[:],
            )

    pending.append(epilogue)


def _emit_pv(nc, ps_o, va, pt, kc, C):
    for qh in range(QH):
        nc.tensor.matmul(
            ps_o[qh][:],
            lhsT=va[:, kc, :],
            rhs=pt[:, qh * 512:(qh + 1) * 512],
            start=(kc == 0), stop=(kc == C - 1),
        )


def build_program(cvec=DENSE_CVEC, loop: int = 1, repeat: int = 1):
    """One SPMD program; head slot s (0..15) covers batch s//2 with
    cvec[s//2] dense chunks."""
    nc = bacc.Bacc("TRN2", target_bir_lowering=False, debug=False,
                   enable_asserts=True, num_devices=N_CORES)
    qT = nc.dram_tensor("qT", [H, D, S], F32R, kind="ExternalInput").ap()
    kT = nc.dram_tensor("kT", [H, D, S], F32R, kind="ExternalInput").ap()
    v = nc.dram_tensor("v", [H, S, D + 1], F32R, kind="ExternalInput").ap()
    out_shape = [H, D, S] if CFG["epi"] == "bcast" else [H, S, D]
    out = nc.dram_tensor("out", out_shape, F32, kind="ExternalOutput").ap()

    with tile.TileContext(nc) as tc:
        with ExitStack() as ctx:
            const_pool = ctx.enter_context(tc.tile_pool(name="const", bufs=1))
            identity = const_pool.tile([128, 128], F32)
            make_identity(nc, identity[:])

            pools = (
                ctx.enter_context(tc.tile_pool(name="qk", bufs=CFG["qk_bufs"])),
                ctx.enter_context(tc.tile_pool(name="va", bufs=CFG["va_bufs"])),
                ctx.enter_context(tc.tile_pool(name="pt", bufs=CFG["pt_bufs"])),
                ctx.enter_context(tc.tile_pool(name="pv", bufs=6)),
                ctx.enter_context(tc.tile_pool(name="ob", bufs=4)),
                ctx.enter_context(tc.tile_pool(name="sc", bufs=6)),
                ctx.enter_context(tc.tile_pool(name="ps_s", bufs=CFG["ps_s_bufs"], space="PSUM")),
                ctx.enter_context(tc.tile_pool(name="ps_o", bufs=CFG["ps_o_bufs"], space="PSUM")),
                ctx.enter_context(tc.tile_pool(name="ps_t", bufs=CFG["ps_t_bufs"], space="PSUM")),
                identity,
            )

            plan = slot_plan(cvec)

            def body(_i=None):
                pending = []
                for _ in range(repeat):
                    for h in range(H):
                        _emit_head(nc, pools, qT, kT, v, out, h,
                                   cvec[plan[h]], pending)
                while pending:
                    pending.pop(0)()

            if loop == 1:
                body()
            else:
                with tc.For_i(0, loop, 1):
                    body()
    nc.compile()
    return nc


def cvec_of(valid_lens):
    vl = np.asarray(valid_lens).astype(np.int64).reshape(B)
    return tuple(int(min(KC, L // 128 + 1)) for L in vl)


def slot_plan(cvec):
    """Per-core slot order: batch ids (each appearing HPC times), heavy and
    light heads interleaved so small heads' serial chains hide under big
    neighbors' ACT backlog. Deterministic in cvec (host and device agree)."""
    pairs = sorted([(cvec[b], b) for b in range(B) for _ in range(HPC)],
                   key=lambda x: (-x[0], x[1]))
    last = pairs.pop()[1]  # smallest head last: shortest serial drain tail
    order = []
    lo, hi = 0, len(pairs) - 1
    while lo <= hi:
        order.append(pairs[lo][1])
        lo += 1
        if lo <= hi:
            order.append(pairs[hi][1])
            hi -= 1
    order.append(last)
    return order


def make_in_maps(queries, keys, values, valid_lens):
    """Per-core inputs: core m's head slot 2b+j holds head (b, 2m+j)."""
    q = np.ascontiguousarray(
        np.asarray(queries, dtype=np.float32)).reshape(B, H, S, D)
    k = np.ascontiguousarray(
        np.asarray(keys, dtype=np.float32)).reshape(B, H, S, D)
    v = np.ascontiguousarray(
        np.asarray(values, dtype=np.float32)).reshape(B, H, S, D)
    vl = np.asarray(valid_lens).astype(np.int64).reshape(B)
    cvec = cvec_of(vl)

    # [B, H, D+1, ...] staging with mask + fold applied per batch.
    km = k.copy()
    va = np.empty((B, H, S, D + 1), np.float32)
    va[..., :D] = v
    va[..., D] = 1.0
    for b in range(B):
        L, C = int(vl[b]), cvec[b]
        km[b, :, L:, :] = 0.0
        if C < KC:
            # Skipped rows all have unnormalized weight exactly 1; fold their
            # V_aug sum into the (masked) last row of the boundary chunk.
            va[b, :, C * 128 - 1, :] += va[b, :, C * 128:, :].sum(axis=1)

    qT = q.transpose(0, 1, 3, 2)   # [B, H, D, S]
    kT = km.transpose(0, 1, 3, 2)

    # slot s of core m holds head (plan[s], 2m + j) where j counts prior
    # occurrences of plan[s] in the plan.
    plan = slot_plan(cvec)
    occ = {}
    slot_heads = []  # (batch, j) per slot
    for b in plan:
        j = occ.get(b, 0)
        occ[b] = j + 1
        slot_heads.append((b, j))

    in_maps = []
    for m in range(N_CORES):
        idx = ([], [])
        for b, j in slot_heads:
            idx[0].append(b)
            idx[1].append(2 * m + j)
        in_maps.append({
            "qT": np.ascontiguousarray(qT[idx[0], idx[1]]),
            "kT": np.ascontiguousarray(kT[idx[0], idx[1]]),
            "v": np.ascontiguousarray(va[idx[0], idx[1]]),
        })
    return in_maps, cvec


def scatter_outputs(results, cvec):
    """Inverse of the head assignment: full [B*H, S, D] from per-core outs."""
    plan = slot_plan(cvec)
    occ = {}
    slot_heads = []
    for b in plan:
        j = occ.get(b, 0)
        occ[b] = j + 1
        slot_heads.append((b, j))
    out = np.empty((B, H, S, D), dtype=np.float32)
    for m in range(N_CORES):
        for s, (b, j) in enumerate(slot_heads):
            r = results[m][s]
            out[b, 2 * m + j] = r.T if CFG["epi"] == "bcast" else r
    return out.reshape(B * H, S, D)


_NC_CACHE = {}


def _get_nc(cvec, loop=1, repeat=1):
    key = (cvec, loop, repeat, tuple(sorted(CFG.items())))
    if key not in _NC_CACHE:
        _NC_CACHE[key] = build_program(cvec, loop, repeat)
    return _NC_CACHE[key]


def kernel(queries, keys, values, valid_lens):
    from concourse.bass_utils import run_bass_kernel_spmd

    in_maps, cvec = make_in_maps(queries, keys, values, valid_lens)
    nc = _get_nc(cvec)
    res = run_bass_kernel_spmd(nc, in_maps, list(range(N_CORES)))
    return scatter_outputs(
        [res.results[m]["out"] for m in range(N_CORES)], cvec)


# ----------------------------------------------------------------------------
# Cached jitted runner (used by test.py for timing; avoids per-call re-trace
# and ships inputs to the devices once).
# ----------------------------------------------------------------------------
_RUNNER_CACHE = {}


def _get_runner(cvec=DENSE_CVEC, loop: int = 1):
    key = (cvec, loop, tuple(sorted(CFG.items())))
    if key in _RUNNER_CACHE:
        return _RUNNER_CACHE[key]

    import jax
    from jax.sharding import Mesh, PartitionSpec, NamedSharding
    from jax.experimental.shard_map import shard_map
    from concourse import bass2jax

    nc = _get_nc(cvec, loop)
    bass2jax.install_neuronx_cc_hook()

    partition_name = (nc.partition_id_tensor.name
                      if nc.partition_id_tensor else None)
    in_names, out_names, out_avals, zero_outs = [], [], [], []
    for alloc in nc.m.functions[0].allocations:
        if not isinstance(alloc, mybir.MemoryLocationSet):
            continue
        name = alloc.memorylocations[0].name
        if alloc.kind == "ExternalInput":
            if name != partition_name:
                in_names.append(name)
        elif alloc.kind == "ExternalOutput":
            out_names.append(name)
            shape = tuple(alloc.tensor_shape)
            dtype = mybir.dt.np(alloc.dtype)
            out_avals.append(jax.core.ShapedArray(shape, dtype))
            zero_outs.append(np.zeros(shape, dtype))
    n_params = len(in_names)
    n_outs = len(out_avals)
    all_in_names = in_names + out_names
    if partition_name is not None:
        all_in_names = all_in_names + [partition_name]

    def _body(*args):
        operands = list(args)
        if partition_name is not None:
            operands.append(bass2jax.partition_id_tensor())
        outs = bass2jax._bass_exec_p.bind(
            *operands,
            out_avals=tuple(out_avals),
            in_names=tuple(all_in_names),
            out_names=tuple(out_names),
            lowering_input_output_aliases=(),
            sim_require_finite=True,
            sim_require_nnan=True,
            nc=nc,
        )
        return tuple(outs)

    devices = jax.devices()[:N_CORES]
    mesh = Mesh(np.asarray(devices), ("core",))
    donate = tuple(range(n_params, n_params + n_outs))
    sharded = jax.jit(
        shard_map(
            _body, mesh=mesh,
            in_specs=(PartitionSpec("core"),) * (n_params + n_outs),
            out_specs=(PartitionSpec("core"),) * n_outs,
            check_rep=False,
        ),
        donate_argnums=donate, keep_unused=True,
    )

    def run(in_maps):
        concat_in = [
            np.concatenate([m[name] for m in in_maps], axis=0)
            for name in in_names
        ]
        concat_zeros = [
            np.zeros((N_CORES * z.shape[0], *z.shape[1:]), z.dtype)
            for z in zero_outs
        ]
        out_arrs = sharded(*concat_in, *concat_zeros)
        return [
            {
                name: np.asarray(out_arrs[i]).reshape(
                    N_CORES, *out_avals[i].shape)[c]
                for i, name in enumerate(out_names)
            }
            for c in range(N_CORES)
        ]

    def make_dev_args(in_maps):
        sh = NamedSharding(mesh, PartitionSpec("core"))
        concat_in = [
            np.concatenate([m[name] for m in in_maps], axis=0)
            for name in in_names
        ]
        dev_in = [jax.device_put(a, sh) for a in concat_in]
        jax.block_until_ready(dev_in)

        def fresh_zeros():
            zs = [jax.device_put(
                np.zeros((N_CORES * z.shape[0], *z.shape[1:]), z.dtype), sh)
                for z in zero_outs]
            jax.block_until_ready(zs)
            return zs

        return dev_in, fresh_zeros

    _RUNNER_CACHE[key] = (run, sharded, make_dev_args, out_names, out_avals, nc)
    return _RUNNER_CACHE[key]


# revision 31
# speedup vs baseline: 1.5542x; 1.0672x over previous
"""Masked dot-product attention on 8 Trainium2 NeuronCores (Bass/Tile).

Problem: B=8, H=16, S=1024, D=64 attention where scores at key positions
k >= valid_lens[b] are masked to 1e-6 (not -inf) before softmax:
masked keys still contribute V with a uniform (unnormalized) weight of
exp(1e-6) ~= 1.

Sharding (SPMD, one program on 8 cores): each core takes 2 heads from EVERY
batch (core m gets heads b*16 + 2m, b*16 + 2m + 1). Since the masked length
is per-batch, every core sees the identical per-slot workload vector
[C_0, C_0, C_1, C_1, ..., C_7, C_7] where C_b = min(8, L_b//128 + 1) is the
number of 128-row key chunks that must be computed densely. The program is
specialized to that vector (compile cached per distinct valid_lens).

Masking, exactly:
  - kT rows with k >= L are zeroed on the host: their scores become exactly 0
    and their unnormalized weight exp(0) = 1 (vs exp(1e-6) in the reference:
    rel diff 1e-6, far below fp32 tolerance).
  - chunks >= C_b are skipped entirely; every skipped row would have weight
    exactly 1, so the host folds sum_{k >= C_b*128} [V[k], 1] into the
    (always masked) last row of the boundary chunk's V_aug. This is exact.

Device pipeline per head slot (fp32; matmuls in fp32r = full PE rate at
free dim >= 256, ~1.6e-4 max rel err measured on HW):
  1. scoresT[k, q] = K @ Q^T per 128-key chunk, as TWO concurrent row-tiled
     matmuls: query half 0 lives on SBUF partitions 0:64 feeding PE rows
     0:63, half 1 on partitions 64:128 feeding rows 64:127 (the contraction
     dim D=64 only fills half the array, so both halves run in parallel —
     measured 291 ns per pair vs 431 ns for one serial matmul). K chunks are
     duplicated across both partition halves (small); Q is not duplicated.
  2. pT = exp(0.125 * scoresT)  (ACT, PSUM->SBUF, scale folded in; the ACT
     engine at 1 elem/lane/cycle is the pipeline's bottleneck engine)
  3. outT[d(+1), q] += V_aug[kc].T @ pT[kc]   (ones-column of V_aug makes
     row 64 the softmax denominator for free), lagging exp by `lag` chunks
     so the PE never blocks the ACT stream
  4. PE-transpose outT to [q, d+1]; DVE reciprocal + per-partition scale; DMA.
"""

from contextlib import ExitStack

import numpy as np

import concourse.bass as bass  # noqa: F401
import concourse.mybir as mybir
import concourse.tile as tile
from concourse import bacc
from concourse.masks import make_identity

F32 = mybir.dt.float32
F32R = mybir.dt.float32r

B, H, S, D = 8, 16, 1024, 64
N_CORES = 8
HPC = H // N_CORES     # heads per (core, batch) = 2
KC = S // 128          # key chunks per full head
QH = S // 512          # query halves
EXPF = mybir.ActivationFunctionType.Exp
SCALE = 1.0 / 8.0      # 1/sqrt(64)

DENSE_CVEC = (KC,) * B

# Tunables (experiment knobs; values are compile-time).
CFG = {
    "lag": 3,          # chunks between exp and its PV consumption
    "pack": "qsplit",  # QK row-packing: qh0 on partitions 0:64, qh1 on 64:128
    "qt_dup": True,
    "pt_bufs": 6,
    "ps_s_bufs": 2,
    "ps_o_bufs": 2,
    "ps_t_bufs": 2,
    "qk_bufs": 3,
    "va_bufs": 3,
    "epi": "pe",       # "pe": PE-transpose epilogue -> out [S, D]
                       # "bcast": gpsimd broadcast divide -> out [D, S],
                       #          transposed on the host during unshard
    "out_ring": "sp",  # HWDGE ring for output stores: "sp" or "act"
}


def _emit_head(nc, pools, qT, kT, v, out, h, C, pending):
    """Emit one head slot with C dense key chunks. `pending` holds deferred
    epilogues (PE transposes) of previous heads, flushed after this head's
    early QK work so the PE never stalls on the DVE evacuation."""
    (qk_pool, va_pool, pt_pool, pv_pool, ob_pool, sc_pool,
     ps_s_pool, ps_o_pool, ps_t_pool, identity) = pools

    pack = CFG["pack"]
    nE = (C + 1) // 2 if pack is True else C
    nO = C // 2 if pack is True else 0
    if pack == "qsplit":
        # qh0 on partitions 0:64, qh1 on 64:128 — Q is NOT duplicated; the
        # (small) K is duplicated instead. Each chunk = 2 concurrent row-tile
        # MMs writing the two banks of its score tile.
        qt = qk_pool.tile([128, 512], F32R, tag="qt")
        nc.sync.dma_start(qt[0:64, :], qT[h][:, 0:512])
        nc.sync.dma_start(qt[64:128, :], qT[h][:, 512:1024])
        kt = qk_pool.tile([128, C * 128], F32R, tag="kt")
        nc.sync.dma_start(kt[0:64, :], kT[h][:, 0:C * 128])
        nc.sync.dma_start(kt[64:128, :], kT[h][:, 0:C * 128])
    elif pack:
        qt = qk_pool.tile([128, S], F32R, tag="qt")
        for qh in range(QH):
            sl = slice(qh * 512, (qh + 1) * 512)
            nc.sync.dma_start(qt[0:64, sl], qT[h][:, sl])
            nc.sync.dma_start(qt[64:128, sl], qT[h][:, sl])
        kt = qk_pool.tile([128, nE * 128], F32R, tag="kt")
        nc.sync.dma_start(
            kt[0:64, :],
            kT[h][:, 0:nE * 256].rearrange("d (i two c) -> d two i c",
                                           two=2, c=128)[:, 0, :, :]
            if C > 1 else kT[h][:, 0:128])
        if nO:
            nc.sync.dma_start(
                kt[64:128, 0:nO * 128],
                kT[h][:, 0:nO * 256].rearrange("d (i two c) -> d two i c",
                                               two=2, c=128)[:, 1, :, :])
    else:
        qt = qk_pool.tile([64, S], F32R, tag="qt")
        for qh in range(QH):
            sl = slice(qh * 512, (qh + 1) * 512)
            nc.sync.dma_start(qt[:, sl], qT[h][:, sl])
        kt = qk_pool.tile([64, C * 128], F32R, tag="kt")
        nc.sync.dma_start(kt[:], kT[h][:, 0:C * 128])
    va = va_pool.tile([128, C, D + 1], F32R, tag="va")
    nc.sync.dma_start(
        va[:], v[h][0:C * 128].rearrange("(kc p) d -> p kc d", p=128))

    ps_o = [ps_o_pool.tile([D + 1, 512], F32, tag="ps_o", name=f"ps_o{qh}")
            for qh in range(QH)]

    def emit_qk_exp(kc):
        """Packed pair (kc, kc+1) when kc even and kc+1 < C; else solo kc."""
        i = kc // 2
        tiles = []
        ps_a = ps_s_pool.tile([128, S], F32, tag="ps_s", name="ps_sa")
        tiles.append(ps_a)
        if pack == "qsplit":
            nc.tensor.matmul(
                ps_a[:, 0:512],
                lhsT=kt[0:64, kc * 128:(kc + 1) * 128],
                rhs=qt[0:64, :],
                start=True, stop=True,
            )
            nc.tensor.matmul(
                ps_a[:, 512:1024],
                lhsT=kt[64:128, kc * 128:(kc + 1) * 128],
                rhs=qt[64:128, :],
                start=True, stop=True,
            )
            pt = pt_pool.tile([128, S], F32R, tag="pt", name="pt0")
            nc.scalar.activation(pt[:], ps_a[:], EXPF, scale=SCALE)
            return [(kc, pt)]
        if not pack:
            for qh in range(QH):
                nc.tensor.matmul(
                    ps_a[:, qh * 512:(qh + 1) * 512],
                    lhsT=kt[:, kc * 128:(kc + 1) * 128],
                    rhs=qt[:, qh * 512:(qh + 1) * 512],
                    start=True, stop=True,
                )
            pt = pt_pool.tile([128, S], F32R, tag="pt", name="pt0")
            nc.scalar.activation(pt[:], ps_a[:], EXPF, scale=SCALE)
            return [(kc, pt)]
        paired = kc + 1 < C
        if paired:
            ps_b = ps_s_pool.tile([128, S], F32, tag="ps_s", name="ps_sb")
            tiles.append(ps_b)
        for qh in range(QH):
            nc.tensor.matmul(
                ps_a[:, qh * 512:(qh + 1) * 512],
                lhsT=kt[0:64, i * 128:(i + 1) * 128],
                rhs=qt[0:64, qh * 512:(qh + 1) * 512],
                start=True, stop=True,
            )
            if paired:
                nc.tensor.matmul(
                    ps_b[:, qh * 512:(qh + 1) * 512],
                    lhsT=kt[64:128, i * 128:(i + 1) * 128],
                    rhs=qt[64:128, qh * 512:(qh + 1) * 512],
                    start=True, stop=True,
                )
        out = []
        for j, ps in enumerate(tiles):
            pt = pt_pool.tile([128, S], F32R, tag="pt", name=f"pt{j}")
            nc.scalar.activation(pt[:], ps[:], EXPF, scale=SCALE)
            out.append((kc + j, pt))
        return out

    pending_pv = []
    first = True
    kc = 0
    while kc < C:
        produced = emit_qk_exp(kc)
        kc += len(produced)
        if first:
            while pending:
                pending.pop(0)()
            first = False
        pending_pv.extend(produced)
        while len(pending_pv) > CFG["lag"]:
            c0, pt0 = pending_pv.pop(0)
            _emit_pv(nc, ps_o, va, pt0, c0, C)
    for c0, pt0 in pending_pv:
        _emit_pv(nc, ps_o, va, pt0, c0, C)

    # Evacuate PSUM on the DVE right away; defer the PE work.
    pvs = []
    for qh in range(QH):
        pv_sb = pv_pool.tile([D + 1, 512], F32, tag="pv")
        nc.vector.tensor_copy(pv_sb[:], ps_o[qh][:])
        pvs.append(pv_sb)

    def epilogue():
        if CFG["epi"] == "bcast":
            for qh in range(QH):
                nc.vector.reciprocal(pvs[qh][D:D + 1, :], pvs[qh][D:D + 1, :])
                rb = ob_pool.tile([64, 512], F32, tag="rb")
                nc.gpsimd.partition_broadcast(rb[:], pvs[qh][D:D + 1, :])
                ot = ob_pool.tile([64, 512], F32, tag="ot")
                nc.vector.tensor_mul(ot[:], pvs[qh][0:D, :], rb[:])
                nc.sync.dma_start(out[h][:, qh * 512:(qh + 1) * 512], ot[:])
            return
        for qh in range(QH):
            ps_t = ps_t_pool.tile([128, 4, D + 1], F32, tag="ps_t")
            for j in range(4):
                nc.tensor.transpose(
                    ps_t[:, j, :],
                    pvs[qh][:, j * 128:(j + 1) * 128],
                    identity[0:D + 1, 0:D + 1],
                )
            recip = sc_pool.tile([128, 4], F32, tag="recip")
            nc.vector.reciprocal(recip[:], ps_t[:, :, D])
            ob = ob_pool.tile([128, 4, D], F32, tag="ob")
            for j in range(4):
                nc.vector.tensor_scalar_mul(
                    ob[:, j, :], ps_t[:, j, 0:D], recip[:, j:j + 1])
            eng = nc.scalar if CFG["out_ring"] == "act" else nc.sync
            eng.dma_start(
                out[h][qh * 512:(qh + 1) * 512, :].rearrange(
                    "(j p) d -> p j d", p=128),
                ob[:],
            )

    pending.append(epilogue)


def _emit_pv(nc, ps_o, va, pt, kc, C):
    for qh in range(QH):
        nc.tensor.matmul(
            ps_o[qh][:],
            lhsT=va[:, kc, :],
            rhs=pt[:, qh * 512:(qh + 1) * 512],
            start=(kc == 0), stop=(kc == C - 1),
        )


def build_program(cvec=DENSE_CVEC, loop: int = 1, repeat: int = 1):
    """One SPMD program; head slot s (0..15) covers batch s//2 with
    cvec[s//2] dense chunks."""
    nc = bacc.Bacc("TRN2", target_bir_lowering=False, debug=False,
                   enable_asserts=True, num_devices=N_CORES)
    qT = nc.dram_tensor("qT", [H, D, S], F32R, kind="ExternalInput").ap()
    kT = nc.dram_tensor("kT", [H, D, S], F32R, kind="ExternalInput").ap()
    v = nc.dram_tensor("v", [H, S, D + 1], F32R, kind="ExternalInput").ap()
    out_shape = [H, D, S] if CFG["epi"] == "bcast" else [H, S, D]
    out = nc.dram_tensor("out", out_shape, F32, kind="ExternalOutput").ap()

    with tile.TileContext(nc) as tc:
        with ExitStack() as ctx:
            const_pool = ctx.enter_context(tc.tile_pool(name="const", bufs=1))
            identity = const_pool.tile([128, 128], F32)
            make_identity(nc, identity[:])

            pools = (
                ctx.enter_context(tc.tile_pool(name="qk", bufs=CFG["qk_bufs"])),
                ctx.enter_context(tc.tile_pool(name="va", bufs=CFG["va_bufs"])),
                ctx.enter_context(tc.tile_pool(name="pt", bufs=CFG["pt_bufs"])),
                ctx.enter_context(tc.tile_pool(name="pv", bufs=6)),
                ctx.enter_context(tc.tile_pool(name="ob", bufs=4)),
                ctx.enter_context(tc.tile_pool(name="sc", bufs=6)),
                ctx.enter_context(tc.tile_pool(name="ps_s", bufs=CFG["ps_s_bufs"], space="PSUM")),
                ctx.enter_context(tc.tile_pool(name="ps_o", bufs=CFG["ps_o_bufs"], space="PSUM")),
                ctx.enter_context(tc.tile_pool(name="ps_t", bufs=CFG["ps_t_bufs"], space="PSUM")),
                identity,
            )

            plan = slot_plan(cvec)

            def body(_i=None):
                pending = []
                for _ in range(repeat):
                    for h in range(H):
                        _emit_head(nc, pools, qT, kT, v, out, h,
                                   cvec[plan[h]], pending)
                while pending:
                    pending.pop(0)()

            if loop == 1:
                body()
            else:
                with tc.For_i(0, loop, 1):
                    body()
    nc.compile()
    return nc


def cvec_of(valid_lens):
    vl = np.asarray(valid_lens).astype(np.int64).reshape(B)
    return tuple(int(min(KC, L // 128 + 1)) for L in vl)


def slot_plan(cvec):
    """Per-core slot order: batch ids (each appearing HPC times), heavy and
    light heads interleaved so small heads' serial chains hide under big
    neighbors' ACT backlog. Deterministic in cvec (host and device agree)."""
    pairs = sorted([(cvec[b], b) for b in range(B) for _ in range(HPC)],
                   key=lambda x: (-x[0], x[1]))
    last = pairs.pop()[1]  # smallest head last: shortest serial drain tail
    order = []
    lo, hi = 0, len(pairs) - 1
    while lo <= hi:
        order.append(pairs[lo][1])
        lo += 1
        if lo <= hi:
            order.append(pairs[hi][1])
            hi -= 1
    order.append(last)
    return order


def make_in_maps(queries, keys, values, valid_lens):
    """Per-core inputs: core m's head slot 2b+j holds head (b, 2m+j)."""
    q = np.ascontiguousarray(
        np.asarray(queries, dtype=np.float32)).reshape(B, H, S, D)
    k = np.ascontiguousarray(
        np.asarray(keys, dtype=np.float32)).reshape(B, H, S, D)
    v = np.ascontiguousarray(
        np.asarray(values, dtype=np.float32)).reshape(B, H, S, D)
    vl = np.asarray(valid_lens).astype(np.int64).reshape(B)
    cvec = cvec_of(vl)

    # [B, H, D+1, ...] staging with mask + fold applied per batch.
    km = k.copy()
    va = np.empty((B, H, S, D + 1), np.float32)
    va[..., :D] = v
    va[..., D] = 1.0
    for b in range(B):
        L, C = int(vl[b]), cvec[b]
        km[b, :, L:, :] = 0.0
        if C < KC:
            # Skipped rows all have unnormalized weight exactly 1; fold their
            # V_aug sum into the (masked) last row of the boundary chunk.
            va[b, :, C * 128 - 1, :] += va[b, :, C * 128:, :].sum(axis=1)

    qT = q.transpose(0, 1, 3, 2)   # [B, H, D, S]
    kT = km.transpose(0, 1, 3, 2)

    # slot s of core m holds head (plan[s], 2m + j) where j counts prior
    # occurrences of plan[s] in the plan.
    plan = slot_plan(cvec)
    occ = {}
    slot_heads = []  # (batch, j) per slot
    for b in plan:
        j = occ.get(b, 0)
        occ[b] = j + 1
        slot_heads.append((b, j))

    in_maps = []
    for m in range(N_CORES):
        idx = ([], [])
        for b, j in slot_heads:
            idx[0].append(b)
            idx[1].append(2 * m + j)
        in_maps.append({
            "qT": np.ascontiguousarray(qT[idx[0], idx[1]]),
            "kT": np.ascontiguousarray(kT[idx[0], idx[1]]),
            "v": np.ascontiguousarray(va[idx[0], idx[1]]),
        })
    return in_maps, cvec


def scatter_outputs(results, cvec):
    """Inverse of the head assignment: full [B*H, S, D] from per-core outs."""
    plan = slot_plan(cvec)
    occ = {}
    slot_heads = []
    for b in plan:
        j = occ.get(b, 0)
        occ[b] = j + 1
        slot_heads.append((b, j))
    out = np.empty((B, H, S, D), dtype=np.float32)
    for m in range(N_CORES):
        for s, (b, j) in enumerate(slot_heads):
            r = results[m][s]
            out[b, 2 * m + j] = r.T if CFG["epi"] == "bcast" else r
    return out.reshape(B * H, S, D)


_NC_CACHE = {}


def _get_nc(cvec, loop=1, repeat=1):
    key = (cvec, loop, repeat, tuple(sorted(CFG.items())))
    if key not in _NC_CACHE:
        _NC_CACHE[key] = build_program(cvec, loop, repeat)
    return _NC_CACHE[key]


def kernel(queries, keys, values, valid_lens):
    from concourse.bass_utils import run_bass_kernel_spmd

    in_maps, cvec = make_in_maps(queries, keys, values, valid_lens)
    nc = _get_nc(cvec)
    res = run_bass_kernel_spmd(nc, in_maps, list(range(N_CORES)))
    return scatter_outputs(
        [res.results[m]["out"] for m in range(N_CORES)], cvec)


# ----------------------------------------------------------------------------
# Cached jitted runner (used by test.py for timing; avoids per-call re-trace
# and ships inputs to the devices once).
# ----------------------------------------------------------------------------
_RUNNER_CACHE = {}


def _get_runner(cvec=DENSE_CVEC, loop: int = 1):
    key = (cvec, loop, tuple(sorted(CFG.items())))
    if key in _RUNNER_CACHE:
        return _RUNNER_CACHE[key]

    import jax
    from jax.sharding import Mesh, PartitionSpec, NamedSharding
    from jax.experimental.shard_map import shard_map
    from concourse import bass2jax

    nc = _get_nc(cvec, loop)
    bass2jax.install_neuronx_cc_hook()

    partition_name = (nc.partition_id_tensor.name
                      if nc.partition_id_tensor else None)
    in_names, out_names, out_avals, zero_outs = [], [], [], []
    for alloc in nc.m.functions[0].allocations:
        if not isinstance(alloc, mybir.MemoryLocationSet):
            continue
        name = alloc.memorylocations[0].name
        if alloc.kind == "ExternalInput":
            if name != partition_name:
                in_names.append(name)
        elif alloc.kind == "ExternalOutput":
            out_names.append(name)
            shape = tuple(alloc.tensor_shape)
            dtype = mybir.dt.np(alloc.dtype)
            out_avals.append(jax.core.ShapedArray(shape, dtype))
            zero_outs.append(np.zeros(shape, dtype))
    n_params = len(in_names)
    n_outs = len(out_avals)
    all_in_names = in_names + out_names
    if partition_name is not None:
        all_in_names = all_in_names + [partition_name]

    def _body(*args):
        operands = list(args)
        if partition_name is not None:
            operands.append(bass2jax.partition_id_tensor())
        outs = bass2jax._bass_exec_p.bind(
            *operands,
            out_avals=tuple(out_avals),
            in_names=tuple(all_in_names),
            out_names=tuple(out_names),
            lowering_input_output_aliases=(),
            sim_require_finite=True,
            sim_require_nnan=True,
            nc=nc,
        )
        return tuple(outs)

    devices = jax.devices()[:N_CORES]
    mesh = Mesh(np.asarray(devices), ("core",))
    donate = tuple(range(n_params, n_params + n_outs))
    sharded = jax.jit(
        shard_map(
            _body, mesh=mesh,
            in_specs=(PartitionSpec("core"),) * (n_params + n_outs),
            out_specs=(PartitionSpec("core"),) * n_outs,
            check_rep=False,
        ),
        donate_argnums=donate, keep_unused=True,
    )

    def run(in_maps):
        concat_in = [
            np.concatenate([m[name] for m in in_maps], axis=0)
            for name in in_names
        ]
        concat_zeros = [
            np.zeros((N_CORES * z.shape[0], *z.shape[1:]), z.dtype)
            for z in zero_outs
        ]
        out_arrs = sharded(*concat_in, *concat_zeros)
        return [
            {
                name: np.asarray(out_arrs[i]).reshape(
                    N_CORES, *out_avals[i].shape)[c]
                for i, name in enumerate(out_names)
            }
            for c in range(N_CORES)
        ]

    def make_dev_args(in_maps):
        sh = NamedSharding(mesh, PartitionSpec("core"))
        concat_in = [
            np.concatenate([m[name] for m in in_maps], axis=0)
            for name in in_names
        ]
        dev_in = [jax.device_put(a, sh) for a in concat_in]
        jax.block_until_ready(dev_in)

        def fresh_zeros():
            zs = [jax.device_put(
                np.zeros((N_CORES * z.shape[0], *z.shape[1:]), z.dtype), sh)
                for z in zero_outs]
            jax.block_until_ready(zs)
            return zs

        return dev_in, fresh_zeros

    _RUNNER_CACHE[key] = (run, sharded, make_dev_args, out_names, out_avals, nc)
    return _RUNNER_CACHE[key]
